# revision 31
# baseline (speedup 1.0000x reference)
"""Multi-head causal attention (B=4, C=2048, E=1024, H=16, D=64) on 8 trn2 cores.

Sharding: core i = (batch b=i//2, head-group g=i%2).  Each core computes its
batch's attention for 8 heads (512 features) and a partial output projection;
the host sums the two partials per batch (W_o split row-wise).

Per-core kernel (all matmuls float32r: full PE rate at N>=256, FP22 operands):
  phase 1: V = x @ Wv_g            -> [tok, 8 heads x (64 feat + ones col)]
           QT/KT per head-pair     -> [128 feat, 2048 tok]   (x.T pre-done on host)
  phase 2: per (head-pair, q-chunk 512, k-block 128):
           S^T = K^T.T @ Q^T       (row-tiled pair, K=64 contraction x 2 heads)
           W^T = exp(S^T / 8)      (one ACT over both heads' psum banks)
           diagonal causal mask    (DVE multiply with host-provided mask)
           hid/rowsum = [V|1].T @ W^T  (M=65 accumulating over k-blocks)
           normalize by 1/rowsum -> hiddenT staged to DRAM
  phase 3: out = hiddenT.T @ Wo_g  (K=512 contraction via 4 chained matmuls)
"""

import numpy as np

import concourse.bass as bass
import concourse.mybir as mybir
import concourse.tile as tile
from concourse.vector_clock import ScopedClock

B, C, E = 4, 2048, 1024
H, D = 16, 64
N_CORES = 8
GF = 512          # features per head-group (8 heads x 64)
HP = 4            # head-pairs per group
QC = 512          # q-chunk width
KB = 128          # k-block width
NQC = C // QC     # 4
NKB = C // KB     # 16
NE = E // 128     # 8 contraction tiles over E
F32 = mybir.dt.float32
F32R = mybir.dt.float32r
BF16 = mybir.dt.bfloat16

_CACHED_NC = None


class PatchedTC(tile.TileContext):
    """This walrus build caps sync waits per instruction (1 for CTRL, ~2 for
    compute ISA structs).  Hoist excess waits onto same-engine NOPs emitted
    just before the instruction (engine streams execute in order, so the
    semantics are identical), and split the end-of-kernel drain's waits
    across single-wait drain instructions."""

    WAIT_CAP = 1

    def _commit_instruction(self, inst, lazy_reg_writes=True):
        si = getattr(inst, "sync_info", None)
        if (
            si is not None
            and len(si.on_wait) > self.WAIT_CAP
            and getattr(inst, "engine", mybir.EngineType.Unassigned)
            != mybir.EngineType.Unassigned
        ):
            waits = list(si.on_wait)
            keep = waits[: self.WAIT_CAP]
            extra = waits[self.WAIT_CAP :]
            si.on_wait[:] = keep
            for w in extra:
                nop = mybir.InstNoOp(
                    name=f"I-nw{self.nc.next_id()}",
                    engine=inst.engine,
                    bass_nofuse=True,
                    sync_info=mybir.SyncInfo(on_wait=[w], on_update=[]),
                )
                super()._commit_instruction(nop, lazy_reg_writes=False)
        return super()._commit_instruction(inst, lazy_reg_writes)

    def _drain_and_barrier(self, tick_clock, wait_clock):
        carrier = self.nc.sync.drain()
        wait_clock.add_sem_waits(
            carrier.ins, ScopedClock({None: tick_clock.global_clock})
        )
        si = carrier.ins.sync_info
        waits = list(si.on_wait) if si is not None else []
        if len(waits) > 1:
            si.on_wait[:] = waits[:1]
            for w in waits[1:]:
                extra = self.nc.sync.drain()
                extra.ins.sync_info = mybir.SyncInfo(on_wait=[w], on_update=[])
        self.nc.all_engine_barrier()
        assert self.sems is not None
        popped = self.nc._tile_sem_poison_stack.pop()
        assert popped is self._sem_poison
        self.nc.clear_and_free_semaphores(list(self.sems.allocated().values()))
        self.nc.all_engine_barrier()


def build_nc():
    nc = bass.Bass("TRN2", target_bir_lowering=False)
    xT = nc.declare_dram_parameter("xT", [E, C], BF16, isOutput=False)
    Wq = nc.declare_dram_parameter("Wq", [E, GF], BF16, isOutput=False)
    Wk = nc.declare_dram_parameter("Wk", [E, GF], BF16, isOutput=False)
    Wv = nc.declare_dram_parameter("Wv", [E, GF], BF16, isOutput=False)
    Wo = nc.declare_dram_parameter("Wo", [GF, E], BF16, isOutput=False)
    msk = nc.declare_dram_parameter("mask", [128, 4 * QC], mybir.dt.bfloat16, isOutput=False)
    out = nc.declare_dram_parameter("out", [C, E], BF16, isOutput=True)

    xT_t = xT.ap().rearrange("(po pi) f -> pi po f", pi=128)    # [128, 8, C]
    Wq_t = Wq.ap().rearrange("(po pi) f -> pi po f", pi=128)    # [128, 8, GF]
    Wk_t = Wk.ap().rearrange("(po pi) f -> pi po f", pi=128)
    Wv_t = Wv.ap().rearrange("(po pi) f -> pi po f", pi=128)
    Wo_t = Wo.ap().rearrange("(po pi) f -> pi po f", pi=128)    # [128, 4, E]

    with PatchedTC(nc) as tc:
        import contextlib

        with contextlib.ExitStack() as ctx:
            consts = ctx.enter_context(tc.tile_pool(name="consts", bufs=1))
            # one PSUM pool (4 banks) shared by the projection / score /
            # out-proj psums, leaving 4 banks so hidA/hidB double-buffer
            stpool = ctx.enter_context(tc.tile_pool(name="stpsum", bufs=2, space="PSUM"))

            xpool = ctx.enter_context(tc.tile_pool(name="xpool", bufs=1))
            vpool = ctx.enter_context(tc.tile_pool(name="vpool", bufs=1))

            # ---- phase 1a: V for all 8 heads, ones column appended per head
            NHEAD = 2  # token-tiles served by the head-start DMA
            with tc.tile_pool(name="wvpool", bufs=1) as wvpool:
                wv_sb = wvpool.tile([128, NE, GF], BF16)
                # Wv first so the V matmuls can chase the xT chunk arrivals
                nc.sync.dma_start(wv_sb[:], Wv_t[:])
                # head-start: all 8 e-chunks for the first NHEAD token-tiles
                # land in one small strided DMA so compute starts ~6us before
                # the bulk xT chunks finish
                xhead = xpool.tile([128, NE, NHEAD * 128], BF16)
                nc.sync.dma_start(xhead[:], xT_t[:, :, 0 : NHEAD * 128])
                xT_sb = xpool.tile([128, NE, C], BF16)
                # token-halves land separately so V-chains t<8 and the first
                # projection chunks unlock after half the bulk transfer
                for h in range(2):
                    for e in range(NE):
                        nc.sync.dma_start(
                            xT_sb[:, e, h * (C // 2) : (h + 1) * (C // 2)],
                            xT_t[:, e, h * (C // 2) : (h + 1) * (C // 2)],
                        )
                mask_sb = consts.tile([128, 4 * QC], mybir.dt.bfloat16)
                nc.sync.dma_start(mask_sb[:], msk.ap())
                v_sb = vpool.tile([128, NKB, 2 * GF], BF16)  # [tok, kb, h*(64V|64ones)]
                ones = v_sb[:].rearrange("p k (h u) -> p k h u", u=128)[:, :, :, 64:128]
                nc.any.memset(ones, 1.0)
                for t in range(NKB):
                    src = xhead if t < NHEAD else xT_sb
                    pv = stpool.tile([128, 2 * QC], F32, tag="st")
                    for e in range(NE):
                        nc.tensor.matmul(
                            pv[:, 0:GF],
                            lhsT=src[:, e, t * 128 : (t + 1) * 128],
                            rhs=wv_sb[:, e, :],
                            start=(e == 0),
                            stop=(e == NE - 1),
                        )
                    dst = v_sb[:, t, :].rearrange("p (h u) -> p h u", u=128)[:, :, 0:64]
                    nc.vector.tensor_copy(dst, pv[:, 0:GF].rearrange("p (h u) -> p h u", u=64))

            # ---- phases 1b + 2: per head-pair projections + attention
            qkpool = ctx.enter_context(tc.tile_pool(name="qkpool", bufs=2))
            wpool = ctx.enter_context(tc.tile_pool(name="wpool", bufs=2))
            hidpool = ctx.enter_context(tc.tile_pool(name="hidpsum", bufs=2, space="PSUM"))
            wtpool = ctx.enter_context(tc.tile_pool(name="wtpool", bufs=3))
            napool = ctx.enter_context(tc.tile_pool(name="napool", bufs=2))
            hfpool = ctx.enter_context(tc.tile_pool(name="hfpool", bufs=1))
            # hidden^T staged in SBUF (not DRAM): [feat-of-pair, hp, tok]
            hf = hfpool.tile([128, HP, C], BF16)

            for hp in range(HP):
                wq_sb = wpool.tile([128, NE, 128], BF16, tag="wq")
                wk_sb = wpool.tile([128, NE, 128], BF16, tag="wk")
                nc.sync.dma_start(wq_sb[:], Wq_t[:, :, hp * 128 : (hp + 1) * 128])
                nc.sync.dma_start(wk_sb[:], Wk_t[:, :, hp * 128 : (hp + 1) * 128])
                # fp32r here: bf16 row-tiled matmul pairs crash the exec unit
                # (NRT_EXEC_UNIT_UNRECOVERABLE); fp32r pairs are stable and the
                # 2 cyc/row fp32r rate over a concurrent pair matches unpaired
                # bf16 anyway.
                qt = qkpool.tile([128, C], F32R, tag="qt")
                kt = qkpool.tile([128, C], F32R, tag="kt")
                for n in range(NQC):
                    pq = stpool.tile([128, 2 * QC], F32, tag="st")
                    for e in range(NE):
                        nc.tensor.matmul(
                            pq[:, 0:QC],
                            lhsT=wq_sb[:, e, :],
                            rhs=xT_sb[:, e, n * QC : (n + 1) * QC],
                            start=(e == 0),
                            stop=(e == NE - 1),
                        )
                    nc.vector.tensor_copy(qt[:, n * QC : (n + 1) * QC], pq[:, 0:QC])
                    pk = stpool.tile([128, 2 * QC], F32, tag="st")
                    for e in range(NE):
                        nc.tensor.matmul(
                            pk[:, 0:QC],
                            lhsT=wk_sb[:, e, :],
                            rhs=xT_sb[:, e, n * QC : (n + 1) * QC],
                            start=(e == 0),
                            stop=(e == NE - 1),
                        )
                    nc.vector.tensor_copy(kt[:, n * QC : (n + 1) * QC], pk[:, 0:QC])

                def emit_norm_half(hp, qc, hidA, hidB, h):
                    # 1/rowsum via exp(-ln(rs)) on ACT: DVE's bit-exact
                    # reciprocal is ~6 cycles/elem and custom DVE ops don't
                    # compile on this toolchain; ln/exp share one table set
                    # (Reciprocal would force an ACT table reload each use).
                    # Both heads' rowsums packed into one 128-partition tile;
                    # processed in q-halves emitted a block apart so the ACT
                    # queue debt per slot stays under the pipeline slack.
                    HQ = QC // 2
                    lo, hi = qc * QC + h * HQ, qc * QC + (h + 1) * HQ
                    sl = slice(h * HQ, (h + 1) * HQ)
                    rs = napool.tile([128, HQ], F32, tag="rs")
                    nc.vector.tensor_copy(rs[0:64, :], hidA[64:128, sl])
                    nc.vector.tensor_copy(rs[64:128, :], hidB[64:128, sl])
                    lnrs = napool.tile([128, HQ], F32, tag="ln")
                    rec = napool.tile([128, HQ], F32, tag="rec")
                    nc.scalar.activation(
                        lnrs[:], rs[:], mybir.ActivationFunctionType.Ln
                    )
                    nc.scalar.activation(
                        rec[:], lnrs[:], mybir.ActivationFunctionType.Exp, scale=-1.0
                    )
                    nc.vector.tensor_tensor(
                        hf[0:64, hp, lo:hi],
                        hidA[0:64, sl],
                        rec[0:64, :],
                        mybir.AluOpType.mult,
                    )
                    nc.vector.tensor_tensor(
                        hf[64:128, hp, lo:hi],
                        hidB[0:64, sl],
                        rec[64:128, :],
                        mybir.AluOpType.mult,
                    )

                def run_attention():
                    def emit_s(qc, kb, nkb, full_exp):
                        # scores + exp + causal mask for one k-block; the S
                        # pair is emitted one block ahead of its AV pair so
                        # the PE has queued work while ACT catches up at
                        # q-chunk starts
                        st = stpool.tile([128, 2 * QC], F32, tag="st")
                        nc.tensor.matmul(
                            st[:, 0:QC],
                            lhsT=kt[0:64, kb * KB : (kb + 1) * KB],
                            rhs=qt[0:64, qc * QC : (qc + 1) * QC],
                            start=True,
                            stop=True,
                        )
                        nc.tensor.matmul(
                            st[:, QC : 2 * QC],
                            lhsT=kt[64:128, kb * KB : (kb + 1) * KB],
                            rhs=qt[64:128, qc * QC : (qc + 1) * QC],
                            start=True,
                            stop=True,
                        )
                        wt = wtpool.tile([128, 2 * QC], BF16, tag="wt")
                        dr = kb - (nkb - 4)
                        HQ = QC // 2
                        if dr >= 2 and not full_exp:
                            # low q-half fully masked: exp only the live
                            # half (ACT is the co-critical engine); the mask
                            # multiply below zeroes the stale dead half
                            nc.scalar.activation(
                                wt[:].rearrange("p (a b) -> p a b", a=2)[:, :, HQ:QC],
                                st[:].rearrange("p (a b) -> p a b", a=2)[:, :, HQ:QC],
                                mybir.ActivationFunctionType.Exp,
                                scale=0.125,
                            )
                        else:
                            nc.scalar.activation(
                                wt[:], st[:], mybir.ActivationFunctionType.Exp, scale=0.125
                            )
                        if dr >= 0:
                            if dr >= 2:
                                # low q-half fully masked, high half partial
                                mm = mask_sb[:, None, dr * QC : (dr + 1) * QC]
                                nc.vector.tensor_tensor(
                                    wt[:].rearrange("p (a b) -> p a b", a=2),
                                    wt[:].rearrange("p (a b) -> p a b", a=2),
                                    mm.to_broadcast((128, 2, QC)),
                                    mybir.AluOpType.mult,
                                )
                            else:
                                # only the low q-half of dr 0/1 needs masking;
                                # the high half is entirely unmasked
                                nc.vector.tensor_tensor(
                                    wt[:].rearrange("p (a b) -> p a b", a=2)[:, :, 0:HQ],
                                    wt[:].rearrange("p (a b) -> p a b", a=2)[:, :, 0:HQ],
                                    mask_sb[:, None, dr * QC : dr * QC + HQ].to_broadcast(
                                        (128, 2, HQ)
                                    ),
                                    mybir.AluOpType.mult,
                                )
                        return wt  # noqa: B023

                    hid = {}
                    norm_q = []

                    def emit_av(qc, kb, nkb, wt):
                        # hidden rows 0:64; rowsum replicated on rows 64:128
                        # (ones columns embedded in v_sb)
                        if kb == 0:
                            hidA = hidpool.tile([128, QC], F32, tag="hidA")
                            hidB = hidpool.tile([128, QC], F32, tag="hidB")
                            hid[qc] = (hidA, hidB)
                        hidA, hidB = hid[qc]
                        nc.tensor.matmul(
                            hidA[:],
                            lhsT=v_sb[:, kb, 2 * hp * 128 : (2 * hp + 1) * 128],
                            rhs=wt[:, 0:QC],
                            start=(kb == 0),
                            stop=(kb == nkb - 1),
                        )
                        nc.tensor.matmul(
                            hidB[:],
                            lhsT=v_sb[:, kb, (2 * hp + 1) * 128 : (2 * hp + 2) * 128],
                            rhs=wt[:, QC : 2 * QC],
                            start=(kb == 0),
                            stop=(kb == nkb - 1),
                        )
                        if kb == nkb - 1:
                            # by emission order these land ~2 blocks into the
                            # next q-chunk's S stream, keeping the ln/exp out
                            # of the ACT queue slots the next AVs wait on
                            norm_q.append((hp, qc, hidA, hidB, 0))
                            norm_q.append((hp, qc, hidA, hidB, 1))
                            del hid[qc]
                        if norm_q:
                            emit_norm_half(*norm_q.pop(0))

                    # software pipeline: AV pair lags the S/exp pair by 2
                    # blocks so the exp latency (~1.2us) is fully hidden
                    # behind queued PE work; wt bufs=3 covers the lag
                    blocks = [
                        (qc, kb, 4 * qc + 4)
                        for qc in range(NQC)
                        for kb in range(4 * qc + 4)
                    ]
                    LAG = 2
                    wts = {}
                    for i, (qc, kb, nkb) in enumerate(blocks):
                        # the first 3 wt-pool buffers (hp 0, blocks 0-2) are
                        # uninitialized SBUF; a NaN there would survive the
                        # mask multiply, so those use the full-width exp
                        wts[i] = emit_s(qc, kb, nkb, full_exp=(hp == 0 and i < 3))
                        j = i - LAG
                        if j >= 0:
                            emit_av(*blocks[j], wts.pop(j))
                    for j in range(len(blocks) - LAG, len(blocks)):
                        emit_av(*blocks[j], wts.pop(j))
                    while norm_q:
                        emit_norm_half(*norm_q.pop(0))

                run_attention()

            # ---- phase 3: out projection, contracting all 512 group features
            with tc.tile_pool(name="opool", bufs=1) as opool, tc.tile_pool(
                name="ostage", bufs=3
            ) as ostage:
                wo_sb = opool.tile([128, HP, E], BF16)
                nc.sync.dma_start(wo_sb[:], Wo_t[:])
                for qb in range(C // 128):
                    so = ostage.tile([128, E], BF16, tag="so")
                    for ec in range(E // QC):
                        po = stpool.tile([128, 2 * QC], F32, tag="st")
                        for f in range(HP):
                            nc.tensor.matmul(
                                po[:, 0:QC],
                                lhsT=hf[:, f, qb * 128 : (qb + 1) * 128],
                                rhs=wo_sb[:, f, ec * QC : (ec + 1) * QC],
                                start=(f == 0),
                                stop=(f == HP - 1),
                            )
                        nc.vector.tensor_copy(
                            so[:, ec * QC : (ec + 1) * QC], po[:, 0:QC]
                        )
                    nc.sync.dma_start(
                        out.ap()[qb * 128 : (qb + 1) * 128, :], so[:]
                    )
    return nc


def _make_mask():
    import ml_dtypes

    m = np.zeros((128, 4, QC), dtype=np.float32)
    for rr in range(4):
        kk = np.arange(128)[:, None]
        qq = np.arange(QC)[None, :]
        m[:, rr, :] = (128 * rr + kk <= qq).astype(np.float32)
    return np.ascontiguousarray(m.reshape(128, 4 * QC)).astype(ml_dtypes.bfloat16)


def make_in_maps(x, W_q, W_k, W_v, W_o):
    import ml_dtypes

    bf16 = ml_dtypes.bfloat16
    mask = _make_mask()
    in_maps = []
    for i in range(N_CORES):
        b, g = i // 2, i % 2
        in_maps.append(
            {
                "xT": np.ascontiguousarray(np.asarray(x)[b].T).astype(bf16),
                "Wq": np.ascontiguousarray(
                    np.asarray(W_q)[:, g * GF : (g + 1) * GF]
                ).astype(bf16),
                "Wk": np.ascontiguousarray(
                    np.asarray(W_k)[:, g * GF : (g + 1) * GF]
                ).astype(bf16),
                "Wv": np.ascontiguousarray(
                    np.asarray(W_v)[:, g * GF : (g + 1) * GF]
                ).astype(bf16),
                "Wo": np.ascontiguousarray(
                    np.asarray(W_o)[g * GF : (g + 1) * GF, :]
                ).astype(bf16),
                "mask": mask,
            }
        )
    return in_maps


def kernel(x, W_q, W_k, W_v, W_o):
    global _CACHED_NC
    from concourse.bass_utils import run_bass_kernel_spmd

    if _CACHED_NC is None:
        _CACHED_NC = build_nc()
    nc = _CACHED_NC

    in_maps = make_in_maps(x, W_q, W_k, W_v, W_o)
    res = run_bass_kernel_spmd(nc, in_maps, core_ids=list(range(N_CORES)))
    out = np.empty((B, C, E), dtype=np.float32)
    for b in range(B):
        out[b] = res.results[2 * b]["out"].astype(np.float32) + res.results[
            2 * b + 1
        ]["out"].astype(np.float32)
    return out

